# revision 1
# baseline (speedup 1.0000x reference)
"""nn_CrossMamba Trainium2 kernel (v2).

Bidirectional Mamba over x = concat(context+seg_c, query+seg_q) (T=4096).
Only the query half of the output is needed, which makes the backward
direction depend on the reversed QUERY alone (2048 causal steps — the
context never reaches it). Sharding: 8 cores = (direction 2) x (batch 2)
x (d_inner half 2); no collectives — each core computes a partial
out-projection over its 512 channels; the host sums partials.

Per-core stream layout (4 chunk slots of TC=1024 columns):
  fwd cores: slots 0..3 = x (context | query); outputs emitted for
    slots 2,3 only (the query half).
  bwd cores: slots 0,1 = zeros, slots 2,3 = reversed query. The zero
    region keeps h == 0, so slot 2 starts the backward scan correctly;
    outputs (slots 2,3) = mamba(reversed query), un-flipped on the host.

Engine budget per chunk (measured ns on HW): DVE runs the 64
tensor_tensor_scan rows (2289 each) plus all-bf16 2x muls (689);
GPSIMD takes dBx muls on no-output slots and the yacc accumulate;
ACT does exp/silu/sigmoid/ln, PSUM->SBUF broadcast copies and hstate
taps; PE does conv-folded in_proj, dbl/dt, B/C row broadcasts and
out_proj. B rows are sign-flipped host-side so dBx = (lns*xc)*(-B)
needs no negation op.
"""

import sys

_TRN_REPO = "/opt/trn_rl_repo"
if _TRN_REPO not in sys.path:
    sys.path.insert(0, _TRN_REPO)

import numpy as np
import ml_dtypes

import concourse.bass as bass
import concourse.mybir as mybir
import concourse.tile as tile
from concourse import bacc
from concourse.bass import ds, ts

F32 = mybir.dt.float32
F32R = mybir.dt.float32r
BF16 = mybir.dt.bfloat16
AF = mybir.ActivationFunctionType
OP = mybir.AluOpType

T = 4096          # total stream (4 chunk slots)
TC = 1024         # time chunk
NCH = T // TC     # 4
OUT0 = 2          # first output chunk slot
TOUT = (NCH - OUT0) * TC   # 2048 output columns
DM = 512          # d_model
DF = 1024         # d_inner full
DH = 512          # d_inner half (per core)
S = 16            # d_state
R = 32            # dt_rank
KC = 4            # d_conv
NKM = DM // 128   # 4  K-tiles for in_proj
NDF = DF // 128   # 8  d-tiles full
NDH = DH // 128   # 4  d-tiles half
NMO = DM // 128   # 4  M-tiles for out_proj


def build_program(stage="full"):
    nc = bacc.Bacc("TRN2", target_bir_lowering=False, debug=False, num_devices=8)

    xT = nc.dram_tensor("xT", [DM, T + KC - 1], BF16, kind="ExternalInput")
    Win_l = nc.dram_tensor("Win_l", [DM, KC * DF + DH], BF16, kind="ExternalInput")
    convb = nc.dram_tensor("convb", [DF, 1], F32, kind="ExternalInput")
    Wx_l = nc.dram_tensor("Wx_l", [DF, R + 2 * S], BF16, kind="ExternalInput")
    Wdt_l = nc.dram_tensor("Wdt_l", [R + 1, DH], F32R, kind="ExternalInput")
    A_h = nc.dram_tensor("A_h", [DH, S], F32, kind="ExternalInput")
    D_h = nc.dram_tensor("D_h", [DH, 1], F32, kind="ExternalInput")
    Wout_l = nc.dram_tensor("Wout_l", [DH, DM], BF16, kind="ExternalInput")
    sel = nc.dram_tensor("sel", [2 * S, 2 * S * 128], F32R, kind="ExternalInput")
    ones_d = nc.dram_tensor("ones_d", [1, T], F32R, kind="ExternalInput")

    if stage == "inproj":
        dbg = nc.dram_tensor("dbg", [DF + DH, T], F32R, kind="ExternalOutput")
    elif stage == "dt":
        dbg = nc.dram_tensor("dbg", [DF, T], F32R, kind="ExternalOutput")
    elif stage == "dbl":
        dbg = nc.dram_tensor("dbg", [R + 2 * S + 1, T], F32R, kind="ExternalOutput")
    elif stage == "scan":
        dbg = nc.dram_tensor("dbg", [DH, TOUT], F32R, kind="ExternalOutput")
    else:
        outT = nc.dram_tensor("outT", [DM, TOUT], F32R, kind="ExternalOutput")

    with tile.TileContext(nc) as tc:
        _emit(nc, tc, stage, locals())
    nc.compile()
    return nc


def _emit(nc, tc, stage, tens):
    xT, Win_l, convb = tens["xT"], tens["Win_l"], tens["convb"]
    Wx_l, Wdt_l, A_h, D_h, Wout_l = (
        tens["Wx_l"], tens["Wdt_l"], tens["A_h"], tens["D_h"], tens["Wout_l"])
    sel_d = tens["sel"]
    ones_d = tens["ones_d"]
    dbg = tens.get("dbg")
    outT = tens.get("outT")

    from contextlib import ExitStack
    ctx = ExitStack()
    with ctx:
        wpool = ctx.enter_context(tc.tile_pool(name="weights", bufs=1))
        xpool = ctx.enter_context(tc.tile_pool(name="xT", bufs=2))
        xcpool = ctx.enter_context(tc.tile_pool(name="xc", bufs=1))
        xchpool = ctx.enter_context(tc.tile_pool(name="xch", bufs=2))
        zpool = ctx.enter_context(tc.tile_pool(name="z", bufs=2))
        dwpool = ctx.enter_context(tc.tile_pool(name="dtw", bufs=1))
        sipool = ctx.enter_context(tc.tile_pool(name="scanin", bufs=2))
        spool = ctx.enter_context(tc.tile_pool(name="scan", bufs=2))
        ypool = ctx.enter_context(tc.tile_pool(name="yacc", bufs=1))
        hpool = ctx.enter_context(tc.tile_pool(name="hstate", bufs=1))
        ps_mm = ctx.enter_context(tc.tile_pool(name="psmm", bufs=2, space="PSUM"))
        ps_bc = ctx.enter_context(tc.tile_pool(name="psbc", bufs=2, space="PSUM"))

        # --- persistent weights in SBUF ---
        w_in = []
        for k in range(NKM):
            t_ = wpool.tile([128, KC * DF + DH], BF16, tag=f"win{k}", name=f"win{k}")
            nc.sync.dma_start(t_[:, :], Win_l[ts(k, 128), :])
            w_in.append(t_)
        w_x = []
        for k in range(NDF):
            t_ = wpool.tile([128, R + 2 * S], BF16, tag=f"wx{k}", name=f"wx{k}")
            nc.sync.dma_start(t_[:, :], Wx_l[ts(k, 128), :])
            w_x.append(t_)
        w_dt = wpool.tile([R + 1, DH], F32R, tag="wdt", name="wdt")
        nc.sync.dma_start(w_dt[:, :], Wdt_l[:, :])
        w_out = []
        for k in range(NDH):
            t_ = wpool.tile([128, DM], BF16, tag=f"wout{k}", name=f"wout{k}")
            nc.sync.dma_start(t_[:, :], Wout_l[ts(k, 128), :])
            w_out.append(t_)
        cb = []
        for k in range(NDF):
            t_ = wpool.tile([128, 1], F32, tag=f"cb{k}", name=f"cb{k}")
            nc.sync.dma_start(t_[:, :], convb[ts(k, 128), :])
            cb.append(t_)
        a_sb = []
        d_sb = []
        for k in range(NDH):
            t_ = wpool.tile([128, S], F32, tag=f"a{k}", name=f"a{k}")
            nc.sync.dma_start(t_[:, :], A_h[ts(k, 128), :])
            a_sb.append(t_)
            t_ = wpool.tile([128, 1], F32, tag=f"dd{k}", name=f"dd{k}")
            nc.sync.dma_start(t_[:, :], D_h[ts(k, 128), :])
            d_sb.append(t_)
        # persistent scan state [128, S] per half d-tile
        hstate = [hpool.tile([128, S], F32, tag=f"hs{k}", name=f"hs{k}")
                  for k in range(NDH)]

        NT = TC // 512  # matmul N-chunks per time chunk

        for c in range(NCH):
            has_out = c >= OUT0
            co = c - OUT0

            # ---- load xT chunk with KC-1 leading halo columns ----
            xt = [xpool.tile([128, TC + KC - 1], BF16, tag=f"xt{k}", name=f"xt{k}")
                  for k in range(NKM)]
            for k in range(NKM):
                nc.sync.dma_start(xt[k][:, :], xT[ts(k, 128), ds(c * TC, TC + KC - 1)])

            # ---- in_proj with conv folded (4 time-shifted matmul blocks);
            # dbl accumulates in PSUM across d-tiles ----
            scanin_dt = sipool.tile([R + 1, TC], F32R, tag="scanin", name="scanin")
            bc_sb = sipool.tile([2 * S, TC], F32R, tag="bcsb", name="bcsb")
            nc.sync.dma_start(scanin_dt[R:R + 1, :], ones_d[:, ds(c * TC, TC)])
            psdb = [ps_mm.tile([R + 2 * S, 512], F32, tag="dblps", name="dblps",
                               bufs=2)
                    for _ in range(NT)]
            xc_t = []
            for j in range(NDF):
                xc = xcpool.tile([128, TC], BF16, tag="xc", name="xc", bufs=2) \
                    if j >= NDH else \
                    xchpool.tile([128, TC], BF16, tag=f"xch{j}", name=f"xch{j}")
                for n in range(NT):
                    psn = ps_mm.tile([128, 512], F32, tag="mm", name="mm")
                    for kk in range(KC):
                        for k in range(NKM):
                            nc.tensor.matmul(
                                psn[:, :],
                                w_in[k][:, ds(kk * DF + j * 128, 128)],
                                xt[k][:, ds(kk + n * 512, 512)],
                                start=(kk == 0 and k == 0),
                                stop=(kk == KC - 1 and k == NKM - 1),
                            )
                    nc.scalar.activation(xc[:, ds(n * 512, 512)], psn[:, :],
                                         AF.Silu, bias=cb[j][:, 0:1])
                xc_t.append(xc)
                if stage == "inproj":
                    nc.sync.dma_start(dbg[ts(j, 128), ds(c * TC, TC)], xc[:, :])
                for n in range(NT):
                    nc.tensor.matmul(
                        psdb[n][:, :], w_x[j][:, 0:R + 2 * S],
                        xc[:, ds(n * 512, 512)],
                        start=(j == 0), stop=(j == NDF - 1),
                    )

            # z tiles (output slots only): M-tiles 8..11 of in_proj
            zg = []
            if has_out and stage == "full":
                for j in range(NDH):
                    z = zpool.tile([128, TC], BF16, tag=f"z{j}", name=f"z{j}")
                    for n in range(NT):
                        psn = ps_mm.tile([128, 512], F32, tag="mm", name="mm")
                        for k in range(NKM):
                            nc.tensor.matmul(
                                psn[:, :],
                                w_in[k][:, ds(KC * DF + j * 128, 128)],
                                xt[k][:, ds(KC - 1 + n * 512, 512)],
                                start=(k == 0), stop=(k == NKM - 1),
                            )
                        nc.scalar.activation(z[:, ds(n * 512, 512)], psn[:, :],
                                             AF.Silu)
                    zg.append(z)

            # ---- collect dbl results from PSUM (DVE casts, small tiles) ----
            for n in range(NT):
                nc.scalar.activation(scanin_dt[0:R, ds(n * 512, 512)],
                                     psdb[n][0:R, :], AF.Copy)
                nc.scalar.activation(bc_sb[:, ds(n * 512, 512)],
                                     psdb[n][R:R + 2 * S, :], AF.Copy)
            if stage == "dbl":
                nc.sync.dma_start(dbg[0:R + 1, ds(c * TC, TC)], scanin_dt[:, :])
                nc.sync.dma_start(dbg[R + 1:R + 1 + 2 * S, ds(c * TC, TC)], bc_sb[:, :])

            # ---- dt path: p = W_dt·dblr + b_dt;  lns = ln(sigmoid(-p)) = -dt
            # w~ = lns * xc  (so dBx = w~ * (-B) = dt*xc*B; B sign-flipped in sel)
            lns_t, w_t = [], []
            for j in range(NDH):
                lns = dwpool.tile([128, TC], BF16, tag=f"lns{j}", name=f"lns{j}")
                for n in range(NT):
                    psn = ps_mm.tile([128, 512], F32, tag="mm", name="mm")
                    nc.tensor.matmul(
                        psn[:, :], w_dt[:, ds(j * 128, 128)],
                        scanin_dt[0:R + 1, ds(n * 512, 512)],
                        start=True, stop=True,
                    )
                    nc.scalar.activation(lns[:, ds(n * 512, 512)], psn[:, :],
                                         AF.Sigmoid, scale=-1.0)
                lns_t.append(lns)
            for j in range(NDH):
                nc.scalar.activation(lns_t[j][:, :], lns_t[j][:, :], AF.Ln)
                w = dwpool.tile([128, TC], BF16, tag=f"w{j}", name=f"w{j}")
                nc.vector.tensor_tensor(w[:, :], lns_t[j][:, :], xc_t[j][:, :],
                                        op=OP.mult)
                w_t.append(w)
                if stage == "dt":
                    nc.sync.dma_start(dbg[ts(j, 128), ds(c * TC, TC)], lns_t[j][:, :])

            if stage in ("inproj", "dbl", "dt"):
                continue

            # ---- scan over states ----
            # two accumulators: even states via GPSIMD into F32R yaccA,
            # odd states via DVE all-bf16 2x adds into yaccB
            yaccA = [ypool.tile([128, TC], F32R, tag=f"ya{j}", name=f"ya{j}")
                     for j in range(NDH)]
            yaccB = [ypool.tile([128, TC], BF16, tag=f"yb{j}", name=f"yb{j}")
                     for j in range(NDH)]
            for s in range(S):
                # broadcast -B_s / C_s rows across 128 partitions via selector
                # matmuls into one [128, TC] PSUM tile each, then one ACT copy
                # to SBUF bf16.
                selb = spool.tile([2 * S, 128], F32R, tag="selb", name="selb", bufs=2)
                nc.sync.dma_start(selb[:, :], sel_d[:, ts(s, 128)])
                pb = ps_bc.tile([128, TC], F32, tag="bc", name="bc")
                for n in range(NT):
                    nc.tensor.matmul(pb[:, ds(n * 512, 512)], selb[:, :],
                                     bc_sb[:, ds(n * 512, 512)],
                                     start=True, stop=True)
                bb_sb = spool.tile([128, TC], BF16, tag="bbsb", name="bbsb", bufs=2)
                nc.scalar.activation(bb_sb[:, :], pb[:, :], AF.Copy)
                if has_out:
                    selc = spool.tile([2 * S, 128], F32R, tag="selc", name="selc",
                                      bufs=2)
                    nc.sync.dma_start(selc[:, :], sel_d[:, ts(S + s, 128)])
                    pc = ps_bc.tile([128, TC], F32, tag="bc", name="bc")
                    for n in range(NT):
                        nc.tensor.matmul(pc[:, ds(n * 512, 512)], selc[:, :],
                                         bc_sb[:, ds(n * 512, 512)],
                                         start=True, stop=True)
                    cc_sb = spool.tile([128, TC], BF16, tag="ccsb", name="ccsb",
                                       bufs=2)
                    nc.scalar.activation(cc_sb[:, :], pc[:, :], AF.Copy)
                for j in range(NDH):
                    dA = spool.tile([128, TC], BF16, tag="dA", name="dA", bufs=3)
                    nc.scalar.activation(dA[:, :], lns_t[j][:, :], AF.Exp,
                                         scale=a_sb[j][:, s:s + 1])
                    dBx = spool.tile([128, TC], BF16, tag="dBx", name="dBx", bufs=3)
                    if has_out:
                        nc.gpsimd.tensor_tensor(dBx[:, :], w_t[j][:, :],
                                                bb_sb[:, :], op=OP.mult)
                    else:
                        nc.vector.tensor_tensor(dBx[:, :], w_t[j][:, :],
                                                bb_sb[:, :], op=OP.mult)
                    h = spool.tile([128, TC], BF16, tag="h", name="h")
                    init = 0.0 if c == 0 else hstate[j][:, s:s + 1]
                    nc.vector.tensor_tensor_scan(h[:, :], dA[:, :], dBx[:, :],
                                                 init, op0=OP.mult, op1=OP.add)
                    if c < NCH - 1:
                        nc.scalar.activation(hstate[j][:, s:s + 1], h[:, TC - 1:TC],
                                             AF.Copy)
                    if has_out:
                        # h*C_s accumulated into yaccA (even s, GPS) or
                        # yaccB (odd s, DVE 2x)
                        acc = yaccA[j] if s % 2 == 0 else yaccB[j]
                        if s < 2:
                            nc.vector.tensor_tensor(acc[:, :], h[:, :],
                                                    cc_sb[:, :], op=OP.mult)
                        else:
                            ym = spool.tile([128, TC], BF16, tag="ym", name="ym",
                                            bufs=2)
                            nc.vector.tensor_tensor(ym[:, :], h[:, :], cc_sb[:, :],
                                                    op=OP.mult)
                            if s % 2 == 0:
                                nc.gpsimd.tensor_tensor(acc[:, :], acc[:, :],
                                                        ym[:, :], op=OP.add)
                            else:
                                nc.vector.tensor_tensor(acc[:, :], acc[:, :],
                                                        ym[:, :], op=OP.add)

            if not has_out:
                continue

            # ---- skip + gate + out_proj ----
            yg = []
            for j in range(NDH):
                y = ypool.tile([128, TC], BF16, tag=f"yg{j}", name=f"yg{j}")
                nc.vector.scalar_tensor_tensor(y[:, :], xc_t[j][:, :],
                                               d_sb[j][:, 0:1], yaccA[j][:, :],
                                               op0=OP.mult, op1=OP.add)
                nc.vector.tensor_tensor(y[:, :], y[:, :], yaccB[j][:, :], op=OP.add)
                if stage == "scan":
                    nc.sync.dma_start(dbg[ts(j, 128), ds(co * TC, TC)], y[:, :])
                    continue
                nc.vector.tensor_tensor(y[:, :], y[:, :], zg[j][:, :], op=OP.mult)
                yg.append(y)
            if stage == "scan":
                continue

            for m in range(NMO):
                for n in range(NT):
                    pso = ps_mm.tile([128, 512], F32, tag="mm", name="mm")
                    for k in range(NDH):
                        nc.tensor.matmul(
                            pso[:, :], w_out[k][:, ds(m * 128, 128)],
                            yg[k][:, ds(n * 512, 512)],
                            start=(k == 0), stop=(k == NDH - 1),
                        )
                    osb = ypool.tile([128, 512], F32R, tag="osb", name="osb", bufs=3)
                    nc.scalar.activation(osb[:, :], pso[:, :], AF.Copy)
                    nc.sync.dma_start(
                        outT[ts(m, 128), ds(co * TC + n * 512, 512)], osb[:, :])


# ---------------------------------------------------------------------------
# host side
# ---------------------------------------------------------------------------

_COMPILED = {}

# selector for the B/C row broadcast: B block negated (absorbs w~ = -dt*xc)
_SEL = np.zeros((2 * S, 2 * S * 128), np.float32)
for _s in range(2 * S):
    _SEL[_s, _s * 128:(_s + 1) * 128] = -1.0 if _s < S else 1.0
_ONES = np.ones((1, T), np.float32)


class _CompiledSpmd:
    def __init__(self, nc, n_cores=8):
        import jax
        from jax.sharding import Mesh, PartitionSpec
        from jax.experimental.shard_map import shard_map
        from concourse.bass2jax import (
            _bass_exec_p, partition_id_tensor, install_neuronx_cc_hook)

        install_neuronx_cc_hook()
        self.jax = jax
        self.nc = nc
        self.n_cores = n_cores
        in_names, out_names, out_avals, zero_outs = [], [], [], []
        partition_name = nc.partition_id_tensor.name if nc.partition_id_tensor else None
        for alloc in nc.m.functions[0].allocations:
            if not isinstance(alloc, mybir.MemoryLocationSet):
                continue
            name = alloc.memorylocations[0].name
            if alloc.kind == "ExternalInput":
                if name != partition_name:
                    in_names.append(name)
            elif alloc.kind == "ExternalOutput":
                shape = tuple(alloc.tensor_shape)
                dtype = mybir.dt.np(alloc.dtype)
                out_avals.append(jax.core.ShapedArray(shape, dtype))
                out_names.append(name)
                zero_outs.append(np.zeros(shape, dtype))
        assert nc.dbg_addr is None
        self.in_names, self.out_names = in_names, out_names
        self.out_avals, self.zero_outs = out_avals, zero_outs
        all_in = list(in_names) + list(out_names)
        if partition_name is not None:
            all_in.append(partition_name)

        def _body(*args):
            operands = list(args)
            if partition_name is not None:
                operands.append(partition_id_tensor())
            return tuple(_bass_exec_p.bind(
                *operands,
                out_avals=tuple(out_avals), in_names=tuple(all_in),
                out_names=tuple(out_names),
                lowering_input_output_aliases=(),
                sim_require_finite=True, sim_require_nnan=True, nc=nc))

        devices = jax.devices()[:n_cores]
        mesh = Mesh(np.asarray(devices), ("core",))
        n_outs = len(out_avals)
        self.fn = jax.jit(
            shard_map(_body, mesh=mesh,
                      in_specs=(PartitionSpec("core"),) * (len(in_names) + n_outs),
                      out_specs=(PartitionSpec("core"),) * n_outs,
                      check_rep=False),
            keep_unused=True)
        self._zero_dev = None

    def run(self, in_maps):
        jax = self.jax
        concat = [np.concatenate([np.asarray(in_maps[c][nm])
                                  for c in range(self.n_cores)], axis=0)
                  for nm in self.in_names]
        if self._zero_dev is None:
            self._zero_dev = [
                jax.device_put(np.zeros((self.n_cores * z.shape[0], *z.shape[1:]),
                                        z.dtype))
                for z in self.zero_outs]
        args = [jax.device_put(a) for a in concat] + self._zero_dev
        outs = self.fn(*args)
        jax.block_until_ready(outs)
        return outs

    def results(self, outs):
        res = []
        for c in range(self.n_cores):
            d = {}
            for i, nm in enumerate(self.out_names):
                d[nm] = np.asarray(outs[i]).reshape(
                    self.n_cores, *self.out_avals[i].shape)[c]
            res.append(d)
        return res


def _get_compiled(stage="full"):
    if stage not in _COMPILED:
        nc = build_program(stage)
        _COMPILED[stage] = _CompiledSpmd(nc, 8)
    return _COMPILED[stage]


def make_in_maps(**inputs):
    """Build the 8 per-core input dicts from full inputs."""
    inp = {k: np.asarray(v, np.float32) for k, v in inputs.items()}
    Lc = inp["context"].shape[1]
    xf = np.concatenate([inp["context"] + inp["seg_context"],
                         inp["query"] + inp["seg_query"]], axis=1)  # [2, T, 512]
    q = inp["query"] + inp["seg_query"]                              # [2, Lq, 512]
    W_in, conv_w, conv_b = inp["W_in"], inp["conv_w"], inp["conv_b"]
    W_x, W_dt, b_dt = inp["W_x"], inp["W_dt"], inp["b_dt"]
    negA = np.exp(inp["A_log"])  # = -A; dA = exp(A*dt) = exp(negA * lns)
    D, W_out = inp["D"], inp["W_out"]
    Win_x, Win_z = W_in[:DF], W_in[DF:]

    in_maps = []
    metas = []
    for core in range(8):
        dirn, b, half = core // 4, (core // 2) % 2, core % 2
        if dirn == 0:
            xb = xf[b]                                   # [T, 512]
        else:
            # zeros for slots 0,1 then the reversed query
            xb = np.concatenate(
                [np.zeros((T - TOUT, DM), np.float32), q[b, ::-1]], axis=0)
        sl = slice(half * DH, (half + 1) * DH)
        # reorder d_inner so this core's half occupies channel blocks 0..3
        idx_half = np.arange(half * DH, (half + 1) * DH)
        idx_oth = np.arange((1 - half) * DH, (2 - half) * DH)
        perm = np.concatenate([idx_half, idx_oth])
        conv_blocks = [np.ascontiguousarray((Win_x * conv_w[:, k:k + 1]).T[:, perm])
                       for k in range(KC)]
        xpad = np.concatenate([np.zeros((DM, KC - 1), np.float32), xb.T], 1)
        m = {
            "xT": xpad.astype(ml_dtypes.bfloat16),
            "Win_l": np.concatenate(
                conv_blocks + [Win_z.T[:, sl]], 1).astype(ml_dtypes.bfloat16),
            "convb": np.ascontiguousarray(conv_b[perm, None]),
            "Wx_l": np.ascontiguousarray(W_x.T[perm]).astype(ml_dtypes.bfloat16),
            "Wdt_l": np.ascontiguousarray(
                np.concatenate([W_dt[sl].T, b_dt[None, sl]], 0)),
            "A_h": np.ascontiguousarray(negA[sl]),
            "D_h": np.ascontiguousarray(D[sl, None]),
            "Wout_l": np.ascontiguousarray(W_out[:, sl].T).astype(ml_dtypes.bfloat16),
            "sel": _SEL,
            "ones_d": _ONES,
        }
        in_maps.append(m)
        metas.append((dirn, b, half))
    return in_maps, metas


def kernel(**inputs):
    Lc = np.asarray(inputs["context"]).shape[1]
    in_maps, metas = make_in_maps(**inputs)
    k = _get_compiled("full")
    outs = k.run(in_maps)
    res = k.results(outs)
    out = np.zeros((2, T - Lc, DM), np.float32)
    acc = {}
    for core, (dirn, b, half) in enumerate(metas):
        acc.setdefault((dirn, b), np.zeros((DM, TOUT), np.float32))
        acc[(dirn, b)] += res[core]["outT"]
    for b in range(2):
        yf = acc[(0, b)].T               # query positions, forward
        yb = acc[(1, b)].T[::-1]         # un-flip backward query outputs
        out[b] = 0.5 * (yf + yb)
    return out.astype(np.float32)

